# revision 3
# baseline (speedup 1.0000x reference)
"""Trainium2 Bass kernel for an involution Bottleneck block (B=2, Cin=256,
Cmid=64, Cout=256, H=W=56, K=15, G=4).

Sharding: 8 cores = 2 batches x 4 H-quarters (14 output rows each). Each core
receives a zero-padded input halo [256, 28, 70] (7 rows/cols each side), so no
inter-core communication is needed (halo compute is redundant).

Per-core pipeline (channels on SBUF partitions, pixels on free dim):
  conv1 1x1 (PE, bf16) -> BN+ReLU (ACT) -> out1 [64, 28x70] bf16
  reduce 1x1 (PE) -> BN+ReLU (ACT) -> r [16, 784] bf16
  per-tap loop (225 taps): span matmul with 16x-expanded Ws columns produces
    the per-pixel kernel already broadcast across the 64 channel partitions
    (PE, two matmuls land pixel-half A at partitions 0:64 and half B at
    64:128); ACT evicts PSUM with the span bias folded in; DVE multiplies by
    the shifted out1 window (bf16 2x mode; a +1-shifted copy of out1 keeps odd
    kx taps 4B-aligned) and accumulates with a row-wise add tree.
  BN+ReLU (ACT) -> conv3 1x1 (PE) -> BN (ACT) -> +residual (DVE) -> ReLU (ACT)
"""

import sys, types
sys.path.insert(0, "/opt/trn_rl_repo")

import numpy as np
import ml_dtypes
from contextlib import ExitStack

import concourse.bass as bass
import concourse.mybir as mybir
import concourse.tile as tile
from concourse import bacc
from concourse.bass import ts
from concourse.bass_utils import run_bass_kernel_spmd

BF16 = mybir.dt.bfloat16
F32 = mybir.dt.float32
AF = mybir.ActivationFunctionType

K = 15
G = 4
GC = 16
PAD = 7
CIN = 256
CMID = 64
RED = 16
COUT = 256
H = 56
W = 56
B = 2
HB = 14          # output rows per core
HP = HB + 2 * PAD  # 28 padded rows
WP = W + 2 * PAD   # 70 padded cols
NP = HP * WP       # 1960
HH = HB // 2       # 7 rows per half-block
NF = HH * W        # 392 pixels per half-block
NV = 10 + K * K    # vecs columns

_PROGRAM = None  # (nc, names) cache


def _build_program():
    nc = bacc.Bacc(None, target_bir_lowering=False, debug=False)
    with tile.TileContext(nc) as tc, ExitStack() as ctx:
        dram = ctx.enter_context(tc.tile_pool(name="dram", bufs=1, space="DRAM"))
        xb_d = dram.tile([CIN, NP], BF16, kind="ExternalInput", name="xb")
        xr_d = dram.tile([COUT, HB * W], F32, kind="ExternalInput", name="xr")
        w1t_d = dram.tile([CIN, CMID], BF16, kind="ExternalInput", name="w1t")
        wrt_d = dram.tile([CMID, RED], BF16, kind="ExternalInput", name="wrt")
        wse_d = dram.tile([RED, K * K * CMID], BF16, kind="ExternalInput", name="wse")
        w3t_d = dram.tile([CMID, COUT], BF16, kind="ExternalInput", name="w3t")
        vec_d = dram.tile([128, NV], F32, kind="ExternalInput", name="vecs")
        y_d = dram.tile([COUT, HB * W], F32, kind="ExternalOutput", name="y")

        wpool = ctx.enter_context(tc.tile_pool(name="weights", bufs=1))
        w1t = wpool.tile([128, 2, CMID], BF16)
        nc.sync.dma_start(out=w1t[:], in_=w1t_d[:].rearrange("(c p) m -> p c m", p=128))
        wrt = wpool.tile([CMID, RED], BF16)
        nc.sync.dma_start(out=wrt[:], in_=wrt_d[:])
        wse = wpool.tile([RED, K * K * CMID], BF16)
        nc.sync.dma_start(out=wse[:], in_=wse_d[:])
        w3t = wpool.tile([CMID, COUT], BF16)
        nc.sync.dma_start(out=w3t[:], in_=w3t_d[:])
        vecs = wpool.tile([128, NV], F32)
        nc.sync.dma_start(out=vecs[:], in_=vec_d[:])

        xpool = ctx.enter_context(tc.tile_pool(name="xin", bufs=1))
        xb = xpool.tile([128, 2, NP], BF16)
        nc.sync.dma_start(out=xb[:], in_=xb_d[:].rearrange("(c p) n -> p c n", p=128))
        xr = xpool.tile([128, 2, HB * W], F32)
        nc.sync.dma_start(out=xr[:], in_=xr_d[:].rearrange("(c p) n -> p c n", p=128))

        opool = ctx.enter_context(tc.tile_pool(name="out1", bufs=1))
        out1p = opool.tile([128, NP], BF16)
        out1q = opool.tile([128, NP], BF16)

        # conv1: out1 = relu(g1 * (W1 @ x) + b1) over all 28x70 padded pixels
        with tc.tile_pool(name="p1", bufs=2, space="PSUM") as p1:
            for j in range(4):
                ps = p1.tile([CMID, 490], F32, tag="ps1")
                nc.tensor.matmul(ps[:], w1t[:, 0, :], xb[:, 0, ts(j, 490)],
                                 start=True, stop=False)
                nc.tensor.matmul(ps[:], w1t[:, 1, :], xb[:, 1, ts(j, 490)],
                                 start=False, stop=True)
                nc.scalar.activation(out1p[0:CMID, ts(j, 490)], ps[:], AF.Relu,
                                     bias=vecs[0:CMID, 1:2], scale=vecs[0:CMID, 0:1])

        # duplicate for half-block B (rows 7..27 at partitions 64:128) and the
        # +1-shifted copy for odd-kx alignment
        nc.sync.dma_start(out=out1p[CMID:128, 0:(HP - HH) * WP],
                          in_=out1p[0:CMID, HH * WP:NP])
        nc.sync.dma_start(out=out1q[:, 0:1469], in_=out1p[:, 1:1470])

        o3 = out1p[:].rearrange("p (h w) -> p h w", w=WP)
        o3q = out1q[:].rearrange("p (h w) -> p h w", w=WP)

        spool = ctx.enter_context(tc.tile_pool(name="stage", bufs=1))
        r_sb = spool.tile([RED, 2 * NF], BF16)

        # reduce: r = relu(gr * (Wr @ out1_central) + br), central 14x56 pixels
        with tc.tile_pool(name="pr", bufs=2, space="PSUM") as pr:
            for hhalf in range(2):
                ps = pr.tile([RED, NF], F32, tag="psr")
                nc.tensor.matmul(ps[:], wrt[:],
                                 o3[0:CMID, PAD + HH * hhalf:PAD + HH * (hhalf + 1), PAD:PAD + W],
                                 start=True, stop=True)
                nc.scalar.activation(r_sb[:, ts(hhalf, NF)], ps[:], AF.Relu,
                                     bias=vecs[0:RED, 3:4], scale=vecs[0:RED, 2:3])

        # span + involution Hadamard accumulation
        acc = None
        with tc.tile_pool(name="sp", bufs=4, space="PSUM") as sp, \
             tc.tile_pool(name="we", bufs=6) as we_pool, \
             tc.tile_pool(name="prod", bufs=4) as prod_pool, \
             tc.tile_pool(name="rs", bufs=3) as rs_pool, \
             tc.tile_pool(name="accp", bufs=3) as acc_pool:
            for ky in range(K):
                rsum = None
                for kx in range(K):
                    k = ky * K + kx
                    ps = sp.tile([128, NF], F32, tag="spanps")
                    lhsT = wse[:, k * CMID:(k + 1) * CMID]
                    nc.tensor.matmul(ps[0:CMID, :], lhsT, r_sb[:, 0:NF],
                                     start=True, stop=True)
                    nc.tensor.matmul(ps[CMID:128, :], lhsT, r_sb[:, NF:2 * NF],
                                     start=True, stop=True)
                    we = we_pool.tile([128, NF], BF16, tag="we")
                    nc.scalar.activation(we[:], ps[:], AF.Identity,
                                         bias=vecs[:, 10 + k:11 + k], scale=1.0)
                    prod = prod_pool.tile([128, NF], BF16, tag="prod")
                    if kx % 2 == 0:
                        src = o3[:, ky:ky + HH, kx:kx + W]
                    else:
                        src = o3q[:, ky:ky + HH, kx - 1:kx - 1 + W]
                    nc.vector.tensor_mul(
                        prod[:].rearrange("p (h w) -> p h w", w=W),
                        we[:].rearrange("p (h w) -> p h w", w=W), src)
                    if rsum is None:
                        rsum = prod
                    else:
                        nr = rs_pool.tile([128, NF], BF16, tag="rs")
                        nc.vector.tensor_add(nr[:], rsum[:], prod[:])
                        rsum = nr
                if acc is None:
                    acc = rsum
                else:
                    na = acc_pool.tile([128, NF], BF16, tag="acc")
                    nc.vector.tensor_add(na[:], acc[:], rsum[:])
                    acc = na

            # out2 = relu(g2 * inv + b2), then unpack halves to [64, 784]
            out2p = spool.tile([128, NF], BF16)
            nc.scalar.activation(out2p[:], acc[:], AF.Relu,
                                 bias=vecs[:, 5:6], scale=vecs[:, 4:5])

        out2f = spool.tile([CMID, 2 * NF], BF16)
        nc.sync.dma_start(out=out2f[:, 0:NF], in_=out2p[0:CMID, :])
        nc.sync.dma_start(out=out2f[:, NF:2 * NF], in_=out2p[CMID:128, :])

        # conv3 + BN3 + residual + relu
        with tc.tile_pool(name="p3", bufs=2, space="PSUM") as p3, \
             tc.tile_pool(name="ypool", bufs=2) as ypool:
            for mc in range(2):
                for nh in range(2):
                    ps = p3.tile([128, NF], F32, tag="ps3")
                    nc.tensor.matmul(ps[:], w3t[:, ts(mc, 128)], out2f[:, ts(nh, NF)],
                                     start=True, stop=True)
                    t3 = ypool.tile([128, NF], F32, tag="t3")
                    nc.scalar.activation(t3[:], ps[:], AF.Identity,
                                         bias=vecs[:, 8 + mc:9 + mc],
                                         scale=vecs[:, 6 + mc:7 + mc])
                    ys = ypool.tile([128, NF], F32, tag="ys")
                    nc.vector.tensor_add(ys[:], t3[:], xr[:, mc, ts(nh, NF)])
                    yr = ypool.tile([128, NF], F32, tag="yr")
                    nc.scalar.activation(yr[:], ys[:], AF.Relu, scale=1.0)
                    nc.sync.dma_start(
                        out=y_d[:].rearrange("(c p) n -> p c n", p=128)[:, mc, ts(nh, NF)],
                        in_=yr[:])

    nc.compile()
    names = dict(xb=xb_d.name, xr=xr_d.name, w1t=w1t_d.name, wrt=wrt_d.name,
                 wse=wse_d.name, w3t=w3t_d.name, vecs=vec_d.name, y=y_d.name)
    return nc, names


def _get_program():
    global _PROGRAM
    if _PROGRAM is None:
        _PROGRAM = _build_program()
    return _PROGRAM


def _bf16(a):
    return np.asarray(a, dtype=np.float32).astype(ml_dtypes.bfloat16)


def kernel(x, W1, g1, b1, Wr, gr, br, Ws, bs, g2, b2, W3, g3, b3,
           _want_results=False, _trace=False):
    x = np.asarray(x, dtype=np.float32)
    nc, names = _get_program()

    # replicated weight/param tensors
    w1t = _bf16(np.asarray(W1).T)                      # [256, 64]
    wrt = _bf16(np.asarray(Wr).T)                      # [64, 16]
    w3t = _bf16(np.asarray(W3).T)                      # [64, 256]
    # wse[rho, k*64 + c] = Ws[g(c)*K*K + k, rho]
    Ws = np.asarray(Ws, dtype=np.float32)              # [900, 16]
    gidx = np.arange(CMID) // GC                       # [64]
    # [K*K, 64, 16] -> transpose to [16, K*K, 64]
    wse = Ws.reshape(G, K * K, RED)[gidx, :, :]        # [64, 225, 16]
    wse = _bf16(np.ascontiguousarray(wse.transpose(2, 1, 0)).reshape(RED, K * K * CMID))

    vecs = np.zeros((128, NV), dtype=np.float32)
    vecs[0:CMID, 0] = g1
    vecs[0:CMID, 1] = b1
    vecs[0:RED, 2] = gr
    vecs[0:RED, 3] = br
    g2e = np.concatenate([np.asarray(g2), np.asarray(g2)])
    b2e = np.concatenate([np.asarray(b2), np.asarray(b2)])
    vecs[:, 4] = g2e
    vecs[:, 5] = b2e
    vecs[:, 6] = np.asarray(g3)[0:128]
    vecs[:, 7] = np.asarray(g3)[128:256]
    vecs[:, 8] = np.asarray(b3)[0:128]
    vecs[:, 9] = np.asarray(b3)[128:256]
    bse = np.asarray(bs, dtype=np.float32).reshape(G, K * K)[gidx, :]  # [64, 225]
    vecs[0:CMID, 10:] = bse
    vecs[CMID:128, 10:] = bse

    # channel layout on partitions: ch = c*128 + p  ->  [p, c, ...]
    def chunk(a):  # [256, ...] -> [128, 2, ...] flattened back to [256, ...] order
        return np.ascontiguousarray(a)

    in_maps = []
    core_geom = []
    for core in range(8):
        b = core // 4
        h0 = (core % 4) * HB
        xpad = np.zeros((CIN, HP, WP), dtype=np.float32)
        lo, hi = h0 - PAD, h0 + HB + PAD
        slo, shi = max(lo, 0), min(hi, H)
        xpad[:, slo - lo:shi - lo, PAD:PAD + W] = x[b, :, slo:shi, :]
        xbc = _bf16(xpad).reshape(CIN, NP)
        xrc = np.ascontiguousarray(x[b, :, h0:h0 + HB, :]).reshape(COUT, HB * W)
        in_maps.append({
            names["xb"]: chunk(xbc),
            names["xr"]: chunk(xrc),
            names["w1t"]: w1t,
            names["wrt"]: wrt,
            names["wse"]: wse,
            names["w3t"]: w3t,
            names["vecs"]: vecs,
        })
        core_geom.append((b, h0))

    res = run_bass_kernel_spmd(nc, in_maps, list(range(8)), trace=_trace)

    y = np.empty((B, COUT, H, W), dtype=np.float32)
    for core, (b, h0) in enumerate(core_geom):
        y[b, :, h0:h0 + HB, :] = res.results[core][names["y"]].reshape(COUT, HB, W)
    if _want_results:
        return y, res
    return y
